# revision 85
# baseline (speedup 1.0000x reference)
"""Single-head attention (B=4, T=4096, D_IN=1024, D_HEAD=D_OUT=64) on 8 TRN2
NeuronCores.

Sharding: core c handles batch b = c//2 and query-half h = c%2 (2048 queries),
computing K/V for the full sequence of its batch redundantly on both cores of
a pair.  x is transposed/permuted AND cast to bf16 on the host (own query-half
columns first), halving HBM traffic.

Device pipeline per core (all matmul operands bf16):
  A. Projections: own-half passes use [Wk|Wq] packed -> psum[128,512]
     (k rows 0:64 evicted via ACT Identity+bias to kt, q rows 64:128 staged
     bf16 and shifted down via identity matmul); V own in [s, o] orientation
     (lhsT = xt chunk, rhs = Wv).  Other-half passes use [Wk|Wv] packed at
     full 128-row array width: k rows 0:64 to kt, v.T rows 64:128 staged bf16
     and PE-transposed back to [s, o] - no half-empty k-only passes.
  B. Scores: 2 matmuls into one psum slab [128 s, 1024 t]; ONE exp instr per
     slab, alternating ACT (true exp) / DVE (Schraudolph int16 bit-trick).
     GPSIMD cannot read PSUM on real HW, so only these two engines carry exp.
  C. AV with folded denominator: v_sb carries a 65th all-ones column, so
     po[t 128, 0:65] += et_chunk.T @ v[s, 0:65] accumulates the output AND
     the softmax denominator (col 64) in one matmul - the baseline's 512
     separate denominator matmuls are gone.  po0 is one [128, 8, 128] f32
     tile (2 banks); po1 is a pair of [128, 4, 128] tiles in the projection
     banks (tag "pa").  AV matmuls are deferred ~12-24 windows behind their
     scores, keeping exp latency off the PE's in-order path and giving the
     PE filler work during exp-bound stretches.
  D. finish per w/half: rec = reciprocal(po[:, :, 64]); osb(bf16) = po * rec;
     DMA out.  The final flush runs half-0's AVs + finish first so its
     divide+DMA hide under half-1's AV matmuls.  v-bias added host-side
     (softmax weights sum to 1).
  Schedule: w0's 32 windows interleave with the 8 x-passes (1 before each
  pass's projections, 3 after); w1's windows run post-pass and rotate a third
  score slab through po0's retired banks.  PSUM: pa 2 banks + ps 2x2-bank
  slabs + po0 2 banks = 8.
"""

import numpy as np
import ml_dtypes

import concourse.bacc as bacc
import concourse.bass as bass
import concourse.mybir as mybir
import concourse.tile as tile
from concourse.bass_utils import run_bass_kernel_spmd

B, T, D_IN, D_HEAD, D_OUT = 4, 4096, 1024, 64, 64
N_CORES = 8
TQ = T // 2          # queries per core
ND = D_IN // 128     # contraction chunks (8)
NDP = ND // 2        # DoubleRow d-pairs (4)
NS = T // 128        # key chunks of 128 (32)
NP = T // 512        # x passes (8); passes 0-3 are the own query half
SCALE = float(1.0 / np.sqrt(np.float32(D_HEAD)))
LOG2E = 1.4426950408889634
EXP_A = SCALE * LOG2E * 128.0          # folds softmax scale into Schraudolph
EXP_B = 127.0 * 128.0 - 7.42

F32 = mybir.dt.float32
I16 = mybir.dt.int16
F32R = mybir.dt.float32r
BF16 = mybir.dt.bfloat16
F8 = mybir.dt.float8e4
EXPF = mybir.ActivationFunctionType.Exp
COPYF = mybir.ActivationFunctionType.Copy
IDENTF = mybir.ActivationFunctionType.Identity
DR = mybir.MatmulPerfMode.DoubleRow
EXP_PATTERN = "AD"


def emit_body(nc, tc, io, dt_mm=None, phases="ABCD", n_iters=None):
    """Emit the per-core kernel body. io: dict of DRAM APs."""
    xt_d, wall_d, id_d = io["xt"], io["wall"], io["ident"]
    bkq_d, bkk_d, out_d = io["bkq"], io["bkk"], io["out"]

    with (
        tc.tile_pool(name="const", bufs=1) as cpool,
        tc.tile_pool(name="xt", bufs=6) as xpool,
        tc.tile_pool(name="proj", bufs=1) as ppool,
        tc.tile_pool(name="stg", bufs=2) as spool,
        tc.tile_pool(name="exp", bufs=44) as epool,
        tc.tile_pool(name="outp", bufs=1) as opool,
        tc.tile_pool(name="psum", bufs=1, space="PSUM") as qpool,
    ):
        # ---- constants ----
        wall_sb = cpool.tile([128, 2560], BF16)
        id_sb = cpool.tile([128, 64], BF16)
        bkq_sb = cpool.tile([128, 1], F32)
        bkk_sb = cpool.tile([64, 1], F32)
        nc.gpsimd.dma_start(bkq_sb[:], bkq_d[:])
        nc.gpsimd.dma_start(bkk_sb[:], bkk_d[:])
        # wall layout: [0:1024]=wkq (8x128), [1024:1536]=wv own (8x64),
        # [1536:2560]=wkv other (8x128)
        wkq_sb = wall_sb[:, 0:1024].rearrange("p (c m) -> p c m", m=128)
        wv_sb = wall_sb[:, 1024:1536].rearrange("p (c m) -> p c m", m=64)
        wkv_sb = wall_sb[:, 1536:2560].rearrange("p (c m) -> p c m", m=128)

        # persistent per-iteration tensors
        kt = ppool.tile([64, T], BF16)             # k[h, s]
        qt = ppool.tile([64, TQ], BF16)            # q[h, t]
        v_sb = ppool.tile([128, NS, 65], BF16)     # v[s, o] + ones col 64
        osb = opool.tile([128, 16, 64], BF16)
        nc.gpsimd.memset(v_sb[:, :, 64], 1.0)      # denominator ones column

        def body():
            xt_tiles = {}

            def load_xt(p, quarters=False):
                xt_t = xpool.tile([128, ND, 512], BF16, tag="xt", name=f"xt{p}")
                src = xt_d[:, p * 512:(p + 1) * 512]
                srcr = src.rearrange("(c p) t -> p c t", p=128)
                nh = 4 if quarters else 2
                for h in range(nh):
                    nc.sync.dma_start(
                        xt_t[:, h * (ND // nh):(h + 1) * (ND // nh), :],
                        srcr[:, h * (ND // nh):(h + 1) * (ND // nh), :])
                xt_tiles[p] = xt_t

            def own_pass(p):
                # [Wk|Wq] packed: k rows 0:64, q rows 64:128
                pkq = qpool.tile([128, 512], F32, tag="pa", bufs=2,
                                 name=f"pkq{p}")
                for d in range(ND):
                    nc.tensor.matmul(pkq[:], wkq_sb[:, d, :],
                                     xt_tiles[p][:, d, :],
                                     start=(d == 0), stop=(d == ND - 1))
                cols = slice(p * 512, (p + 1) * 512)
                stg = spool.tile([128, 512], BF16, tag="stg", name=f"stg{p}")
                nc.scalar.activation(kt[:, cols], pkq[0:64, :], IDENTF,
                                     bias=bkq_sb[0:64])
                nc.scalar.activation(stg[64:128, :], pkq[64:128, :], IDENTF,
                                     bias=bkq_sb[64:128])
                return stg

            def v_own(p):
                pv = qpool.tile([128, 4, 64], F32, tag="pa", bufs=2,
                                name=f"pv{p}")
                for sc in range(4):
                    for d in range(ND):
                        nc.tensor.matmul(
                            pv[:, sc, :],
                            xt_tiles[p][:, d, sc * 128:(sc + 1) * 128],
                            wv_sb[:, d, :],
                            start=(sc == 0 and d == 0),
                            stop=(sc == 3 and d == ND - 1))
                nc.vector.tensor_copy(v_sb[:, p * 4:(p + 1) * 4, 0:64],
                                      pv[:])
                return stg

            def q_fix(p, stg):
                # shift q rows from partitions 64:128 down to 0:64 on the PE
                pqf = qpool.tile([64, 512], F32, tag="pa", bufs=2,
                                 name=f"pqf{p}")
                nc.tensor.matmul(pqf[:], id_sb[64:128, :], stg[64:128, :],
                                 start=True, stop=True)
                nc.scalar.activation(qt[:, p * 512:(p + 1) * 512], pqf[:],
                                     COPYF)

            def kv_pass(p):
                # fp8 DoubleRow: k rows 0:64, v.T rows 64:128
                pkv = qpool.tile([128, 512], F32, tag="pa", bufs=2,
                                 name=f"pkv{p}")
                for d in range(ND):
                    nc.tensor.matmul(pkv[:], wkv_sb[:, d, :],
                                     xt_tiles[p][:, d, :],
                                     start=(d == 0), stop=(d == ND - 1))
                cols = slice(p * 512, (p + 1) * 512)
                nc.scalar.activation(kt[:, cols], pkv[0:64, :], IDENTF,
                                     bias=bkk_sb[:])
                vt = spool.tile([128, 512], BF16, tag="vt", name=f"vt{p}")
                nc.scalar.activation(vt[64:128, :], pkv[64:128, :], COPYF)
                ptr = qpool.tile([128, 4, 64], BF16, tag="pa", bufs=2,
                                 name=f"ptr{p}")
                for j in range(4):
                    nc.tensor.matmul(ptr[:, j, :],
                                     vt[64:128, j * 128:(j + 1) * 128],
                                     id_sb[64:128, :],
                                     start=(j == 0), stop=(j == 3),
                                     is_transpose=True)
                nc.vector.tensor_copy(v_sb[:, p * 4:(p + 1) * 4, 0:64],
                                      ptr[:])

            pos = {}
            ets = {}
            pending = []
            exp_n = [0]

            def exp_engine(n_half):
                # alternate exp instrs between ACT (true exp) and DVE
                # (Schraudolph int16 bit-trick)
                i = exp_n[0]
                exp_n[0] += 1
                if i < 8:
                    return "D"
                return EXP_PATTERN[i % len(EXP_PATTERN)]

            def do_exp(et, ps, eng):
                if eng == "A":
                    nc.scalar.activation(et[:], ps[:], EXPF, scale=SCALE)
                else:
                    nc.vector.tensor_scalar(et.bitcast(I16), ps[:], EXP_A,
                                            EXP_B,
                                            op0=mybir.AluOpType.mult,
                                            op1=mybir.AluOpType.add)

            def get_po(w):
                if w not in pos:
                    if w == 0:
                        pos[w] = qpool.tile([128, 8, 128], F32, tag="po0",
                                            bufs=1, name="po0")
                    else:
                        # reuse the projection psum banks
                        pos[w] = [
                            qpool.tile([128, 4, 128], F32, tag="pa", bufs=2,
                                       name=f"po{w}{h}")
                            for h in range(2)]
                return pos[w]

            def po_region(w, tc):
                if w == 0:
                    return pos[0][:, tc, 0:65]
                return pos[1][tc // 4][:, tc % 4, 0:65]

            def av_mms(w, s, et, tc0, ntc):
                for jj in range(ntc):
                    tc = tc0 + jj
                    nc.tensor.matmul(po_region(w, tc),
                                     et[:, jj * 128:(jj + 1) * 128],
                                     v_sb[:, s, 0:65],
                                     start=(s == 0 and tc % 4 == 0),
                                     stop=(s == NS - 1 and tc % 4 == 3))

            def flush_pending(w=None):
                keep = []
                for item in pending:
                    if w is None or item[0] == w:
                        av_mms(*item)
                    else:
                        keep.append(item)
                pending[:] = keep

            limit = [6]

            def defer_av(item):
                while len(pending) >= limit[0]:
                    av_mms(*pending.pop(0))
                pending.append(item)

            ph_n = [0]

            po0_free = [False]

            def ps_tile(w):
                # w1 phase rotates in the retired po0 bank pair as a third
                # score slab (3-deep exp pipeline) once finish_w0 released it
                if w == 1 and po0_free[0]:
                    i = ph_n[0] % 3
                    ph_n[0] += 1
                    if i == 2:
                        return qpool.tile([128, 1024], F32, tag="po0",
                                          bufs=1, name=f"px{ph_n[0]}"), True
                return qpool.tile([128, 1024], F32, tag="ps", bufs=2,
                                  name=f"ps{w}_{ph_n[0]}"), False

            def win_half(w, s, half):
                # startup-only: uses half of a slab
                ps = qpool.tile([128, 1024], F32, tag="ps", bufs=2,
                                name=f"ph{w}_{s}_{half}")
                col0 = w * 1024 + half * 512
                nc.tensor.matmul(ps[:, 0:512], kt[:, s * 128:(s + 1) * 128],
                                 qt[:, col0:col0 + 512], start=True, stop=True)
                et = epool.tile([128, 512], BF16, tag="et",
                                name=f"eh{w}_{s}_{half}")
                do_exp(et, ps[:, 0:512], exp_engine(1))
                get_po(w)
                defer_av((w, s, et, half * 4, 4))

            def win(w, s, split=False):
                ps, is_po0 = ps_tile(w)
                col0 = w * 1024
                nc.tensor.matmul(ps[:, 0:512], kt[:, s * 128:(s + 1) * 128],
                                 qt[:, col0:col0 + 512], start=True, stop=True)
                nc.tensor.matmul(ps[:, 512:1024],
                                 kt[:, s * 128:(s + 1) * 128],
                                 qt[:, col0 + 512:col0 + 1024],
                                 start=True, stop=True)
                et = epool.tile([128, 1024], BF16, tag="et",
                                name=f"et{w}_{s}")
                if isinstance(split, str):
                    do_exp(et, ps, split)
                    exp_engine(2)
                elif split:
                    # latency-critical: halves on ACT and DVE in parallel
                    do_exp(et[:, 0:512], ps[:, 0:512], "A")
                    do_exp(et[:, 512:1024], ps[:, 512:1024], "D")
                    exp_engine(2)
                else:
                    do_exp(et, ps, exp_engine(2))
                get_po(w)
                defer_av((w, s, et, 0, 8))

            def finish_w0():
                rec = opool.tile([128, 8], F32, tag="rec", bufs=2,
                                 name="rec0")
                nc.vector.reciprocal(rec[:], pos[0][:, :, 64])
                odst = out_d.rearrange("(j p) o -> p j o", p=128)
                nc.vector.tensor_mul(
                    osb[:, 0:8, :], pos[0][:, :, 0:64],
                    rec[:, :, None].broadcast_to([128, 8, 64]))
                nc.sync.dma_start(odst[:, 0:8, :], osb[:, 0:8, :])

            def finish_w1():
                odst = out_d.rearrange("(j p) o -> p j o", p=128)
                for h in range(2):
                    rec = opool.tile([128, 4], F32, tag="rec", bufs=2,
                                     name=f"rec1{h}")
                    nc.vector.reciprocal(rec[:], pos[1][h][:, :, 64])
                    nc.vector.tensor_mul(
                        osb[:, 8 + 4 * h:12 + 4 * h, :],
                        pos[1][h][:, :, 0:64],
                        rec[:, :, None].broadcast_to([128, 4, 64]))
                    nc.sync.dma_start(odst[:, 8 + 4 * h:12 + 4 * h, :],
                                      osb[:, 8 + 4 * h:12 + 4 * h, :])

            # ---- emission schedule ----
            # constants + first two x passes, interleaved on the sync ring so
            # compute can start as soon as wkq + xt0 quarter 0 arrive
            nc.sync.dma_start(wall_sb[:, 0:256], wall_d[:, 0:256])
            xt_tiles[0] = xpool.tile([128, ND, 512], BF16, tag="xt", name="xt0")
            xt_tiles[1] = xpool.tile([128, ND, 512], BF16, tag="xt", name="xt1")
            for p in range(2):
                srcr = xt_d[:, p * 512:(p + 1) * 512].rearrange(
                    "(c p) t -> p c t", p=128)
                for h in range(4):
                    nc.sync.dma_start(
                        xt_tiles[p][:, h * 2:(h + 1) * 2, :],
                        srcr[:, h * 2:(h + 1) * 2, :])
                    if p == 0 and h == 0:
                        nc.sync.dma_start(wall_sb[:, 256:1024],
                                          wall_d[:, 256:1024])
                if p == 0:
                    nc.sync.dma_start(wall_sb[:, 1024:1536],
                                      wall_d[:, 1024:1536])
                    nc.sync.dma_start(id_sb[:], id_d[:])
            stg0 = own_pass(0)
            v_own(0)
            q_fix(0, stg0)
            for s in range(0, 4):
                win_half(0, s, 0)
            stg1 = own_pass(1)
            v_own(1)
            q_fix(1, stg1)
            for s in range(0, 4):
                win_half(0, s, 1)
            # passes 2-7 interleaved with w0 windows
            for p in range(2, NP):
                load_xt(p, quarters=(p < 4))
                if p == 2:
                    nc.sync.dma_start(wall_sb[:, 1536:2560],
                                      wall_d[:, 1536:2560])
                for s in range(4 * (p - 2) + 4, 4 * (p - 2) + 5):
                    win(0, s)
                if p < 4:
                    stgp = own_pass(p)
                    v_own(p)
                    q_fix(p, stgp)
                else:
                    kv_pass(p)
                for s in range(4 * (p - 2) + 5, 4 * (p - 2) + 8):
                    win(0, s)
            for s in range(28, NS):
                win(0, s)
            flush_pending(0)
            finish_w0()
            po0_free[0] = True
            # w1 windows run after the passes; po1 reuses the "pa" banks
            limit[0] = 24
            for s in range(0, NS):
                win(1, s)
            # flush half-0 AVs, start its finish while half-1 AVs run
            odst = out_d.rearrange("(j p) o -> p j o", p=128)
            for item in pending:
                av_mms(item[0], item[1], item[2][:, 0:512], 0, 4)
            rec0 = opool.tile([128, 4], F32, tag="rec", bufs=2, name="rec10")
            nc.vector.reciprocal(rec0[:], pos[1][0][:, :, 64])
            nc.vector.tensor_mul(
                osb[:, 8:12, :], pos[1][0][:, :, 0:64],
                rec0[:, :, None].broadcast_to([128, 4, 64]))
            nc.sync.dma_start(odst[:, 8:12, :], osb[:, 8:12, :])
            for item in pending:
                av_mms(item[0], item[1], item[2][:, 512:1024], 4, 4)
            pending.clear()
            rec1 = opool.tile([128, 4], F32, tag="rec", bufs=2, name="rec11")
            nc.vector.reciprocal(rec1[:], pos[1][1][:, :, 64])
            nc.vector.tensor_mul(
                osb[:, 12:16, :], pos[1][1][:, :, 0:64],
                rec1[:, :, None].broadcast_to([128, 4, 64]))
            nc.sync.dma_start(odst[:, 12:16, :], osb[:, 12:16, :])

        if n_iters is None:
            body()
        else:
            with tc.For_i(0, n_iters, 1) as _i:
                body()


def build_program(dt_mm=None, phases="ABCD", n_iters=None):
    nc = bacc.Bacc("TRN2", target_bir_lowering=False, debug=False,
                   num_devices=N_CORES)
    io = {
        "xt": nc.dram_tensor("xt", [D_IN, T], BF16, kind="ExternalInput").ap(),
        "wall": nc.dram_tensor("wall", [128, 2560], BF16, kind="ExternalInput").ap(),
        "ident": nc.dram_tensor("ident", [128, 64], BF16, kind="ExternalInput").ap(),
        "bkq": nc.dram_tensor("bkq", [128, 1], F32, kind="ExternalInput").ap(),
        "bkk": nc.dram_tensor("bkk", [64, 1], F32, kind="ExternalInput").ap(),
        "out": nc.dram_tensor("out", [TQ, D_OUT], BF16, kind="ExternalOutput").ap(),
    }
    with tile.TileContext(nc) as tc:
        emit_body(nc, tc, io, dt_mm, phases=phases, n_iters=n_iters)
    nc.compile()
    return nc


_PROGRAM_CACHE = {}


def get_program(dt_mm=None):
    key = str(dt_mm)
    if key not in _PROGRAM_CACHE:
        _PROGRAM_CACHE[key] = build_program(dt_mm)
    return _PROGRAM_CACHE[key]


def make_in_maps(x, Wk, bk, Wq, bq, Wv, bv):
    bf = ml_dtypes.bfloat16
    x = np.asarray(x, dtype=np.float32)

    def blk(Wm):
        m = Wm.shape[1]
        return Wm.astype(np.float32).reshape(ND, 128, m).transpose(1, 0, 2) \
            .reshape(128, ND * m)
    wall = np.ascontiguousarray(np.concatenate(
        [blk(np.concatenate([Wk, Wq], axis=1)),
         blk(Wv),
         blk(np.concatenate([Wk, Wv], axis=1))],
        axis=1)).astype(bf)
    ident = np.zeros((128, 64), dtype=np.float32)
    ident[0:64, :] = np.eye(64)
    ident[64:128, :] = np.eye(64)
    ident = ident.astype(bf)
    bkq = np.concatenate([bk, bq]).astype(np.float32).reshape(128, 1)
    bkk = np.asarray(bk, dtype=np.float32).reshape(64, 1)
    in_maps = []
    for c in range(N_CORES):
        b, half = c // 2, c % 2
        xb = x[b]
        own = xb[half * TQ:(half + 1) * TQ].T
        other = xb[(1 - half) * TQ:(2 - half) * TQ].T
        xt = np.ascontiguousarray(
            np.concatenate([own, other], axis=1)).astype(bf)
        in_maps.append({"xt": xt, "wall": wall, "ident": ident,
                       "bkq": bkq, "bkk": bkk})
    return in_maps


def assemble(results, bv):
    out = np.empty((B, T, D_OUT), dtype=np.float32)
    bv = np.asarray(bv, dtype=np.float32)
    for c in range(N_CORES):
        b, half = c // 2, c % 2
        out[b, half * TQ:(half + 1) * TQ, :] = \
            results[c]["out"].astype(np.float32) + bv
    return out


def kernel(x, Wk, bk, Wq, bq, Wv, bv):
    nc = get_program()
    in_maps = make_in_maps(x, Wk, bk, Wq, bq, Wv, bv)
    res = run_bass_kernel_spmd(nc, in_maps, list(range(N_CORES)))
    return assemble(res.results, bv)
